# revision 1
# baseline (speedup 1.0000x reference)
"""DeepFactor (K relu-LSTM branches + shared Dense head) on 8 trn2 NeuronCores.

Sharding: the K=10 factor branches are expert-split across cores, 2 slots
per core (16 slots = 10 real + 6 zero-padded; zero weights keep the padded
slot's state identically 0 so padding is exact). Every core runs the same
SPMD program over the full batch B=32.

The per-step recurrence is latency-bound on the PE->ACT->DVE->PE loop when
sigmoids run on the scalar engine. This kernel removes ACT from the loop
entirely with an exp-free sigmoid evaluated on the DVE via custom ops:

    sigma(z) ~= recip_1NR(1 + w^256),   w = 1 - z/beta

The (1 - z/beta) affine map is folded into the f|i|o weights (the matmul
emits w directly; the exact +1.0 rides a dedicated ones-row of the
augmented input so fp16 weight quantization stays relative). w^256 is 8
squarings (POW256 op, one instruction for the f|i|o gate block) and
recip_1NR is the BITWISE_NOT exponent-flip seed plus one Newton step whose
constant absorbs the NR undershoot (fused with the gate multiply in
SIGR_MUL / SIGR_MUL_RELU). Max |sigma_hat - sigma| ~= 1.6e-3, near-zero
mean; end-to-end rel err vs the jax reference ~= 2.9e-3 (tol 2e-2).

Per step, gate columns f|i|o|c in one PSUM tile (PSUM accumulation groups
are bank-scoped, so each gate's start=True x-projection matmul stays
adjacent to its stop=True recurrent matmul):
  PE :  z_g  = LX_g.T @ [x_t;1;1]  (start) + LH_g.T @ h  (stop)
  DVE:  v  = z[f|i|o]^256                               POW256
        t1 = sigr(v_i) * relu(zc)                       SIGR_MUL_RELU
        t2 = sigr(v_f) * c                              SIGR_MUL
        c' = t1 + t2
        h  = sigr(v_o) * relu(c')                       SIGR_MUL_RELU
  PE :  y_t = h.T @ [Wd;Wd]   (one PSUM column; sums both k slots)
Host gathers: y = (sum over cores of Y)/K + bd.
"""

import os
from contextlib import ExitStack

import numpy as np

import concourse.bass as bass
import concourse.tile as tile
from concourse import bacc, mybir
from concourse.bass_utils import run_bass_kernel_spmd

# Problem dims (hardcoded per contract)
B, T, D, U, K = 32, 1024, 32, 64, 10
NCORES = 8
CHUNK_STEPS = int(os.environ.get("KERNEL_CHUNK_STEPS", "64"))

FP16 = os.environ.get("KERNEL_FP16", "1") == "1"
NS = int(os.environ.get("KERNEL_NS", "2"))  # phase-shifted batch slices
H_DB = os.environ.get("KERNEL_H_DB", "1") == "1"  # double-buffer h state
Z_BUFS = int(os.environ.get("KERNEL_Z_BUFS", "2"))
V_BUFS = int(os.environ.get("KERNEL_V_BUFS", "2"))
Y_MM = os.environ.get("KERNEL_Y_MM", "1") == "1"
ADD_ENGINE = os.environ.get("KERNEL_ADD_ENGINE", "gpsimd")  # vector | gpsimd
# split pow into i-gate and f|o parts: pow_i starts right after the i-gate
# matmuls, and pow_fo's busy time drains pow_i's ack before t1 reads it
SPLIT_POW = os.environ.get("KERNEL_SPLIT_POW", "0") == "1"
# c-gate in its own PSUM tile: the pow op and t1 then read different PSUM
# tiles, avoiding serialized second reads of one tile
SPLIT_Z = os.environ.get("KERNEL_SPLIT_Z", "0") == "1"

# sigma_hat constants (fit vs true sigmoid; see module docstring). The
# Newton constant TNR absorbs the NR undershoot correction (T = 2+delta).
# (1 - z/beta) is folded into the f|i|o weights, so the matmul emits w
# directly and the POW op is 8 pure squarings (n=256).
SIG_BETA = 256.7562003289679
SIG_C0SEED = -0.23594391924053412
SIG_TNR = 2.00162127342384
D_AUG = D + 2  # x rows + exact-1.0 row + (-b/beta) row

# gate order in the reference weights (Keras): i|f|c|o
_REF_GATE_SLICE = {"i": 0, "f": 1, "c": 2, "o": 3}
# our gate order: i|f|o (sigma_hat block; i first so pow_i can start as soon
# as the i-gate matmuls land) then c (relu'd on DVE)
_OUR_GATES = ["i", "f", "o", "c"]


# --- custom DVE ops (registered into concourse.dve_ops at import) -----------
def _register_custom_ops():
    from concourse import dve_ops
    from concourse.dve_spec import (
        Spec, Src0, Src1, C0, C1, One, Zero, AluOp, Bin, lower, maxx,
        _has_src1,
    )
    from concourse.dve_uop import DveOpSpec

    if "DF_POW256_ANT" in dve_ops._SUB_OPCODE_FOR_NAME:
        return  # already registered in this process

    def _pow256_ref(in0, in1, s0, s1, imm2):
        v = in0.astype(np.float32)
        for _ in range(8):
            v = (v * v).astype(np.float32)
        return v

    v = Src0
    for _ in range(8):
        v = v * v
    pow256_spec = Spec(body=v, reference=_pow256_ref)

    def _sig_core(src1_term):
        # sigma_hat(z)*x = y0*(C1 - d*y0) * x, d = 1+v, seed y0 = ~bits(d)*C0
        d = One + Src0
        nb = Bin(AluOp.BITWISE_NOT, d, d)
        y0 = nb * C0
        y1 = y0 * (C1 - d * y0)
        return y1 * src1_term

    def _sigr_ref(relu):
        def ref(in0, in1, s0, s1, imm2):
            d = (1.0 + in0).astype(np.float32)
            nb = (~d.view(np.int32)).view(np.float32)
            y0 = (nb * np.float32(s0)).astype(np.float32)
            y1 = (y0 * (np.float32(s1) - d * y0)).astype(np.float32)
            t = np.maximum(in1, 0) if relu else in1
            return (y1 * t).astype(np.float32)
        return ref

    sigr_mul_spec = Spec(body=_sig_core(Src1), reference=_sigr_ref(False))
    sigr_mul_relu_spec = Spec(
        body=_sig_core(maxx(Src1, Zero)), reference=_sigr_ref(True)
    )

    ops = []
    for name, spec in (
        ("DF_POW256_ANT", pow256_spec),
        ("DF_SIGR_MUL_ANT", sigr_mul_spec),
        ("DF_SIGR_MUL_RELU_ANT", sigr_mul_relu_spec),
    ):
        row = dve_ops._CUSTOM_DVE_ROW_BASE + len(dve_ops.OPS)
        shas = {}
        for ver in ("v3", "v4"):
            uops = lower(spec, ver=ver)
            s = DveOpSpec(
                name=name, opcode=row, uops=uops, rd1_en=_has_src1(spec)
            )
            shas[ver] = s.sha(ver)
        op = dve_ops.DveOp(name, spec, subdim=False, uops_sha=shas)
        dve_ops.OPS.append(op)
        dve_ops.CUSTOM_DVE_SPECS[name] = spec
        dve_ops._SUB_OPCODE_FOR_NAME[name] = row
        ops.append(op)
    return ops


_register_custom_ops()


def _np_dt():
    return np.float16 if FP16 else np.float32


def _mm_dt():
    return mybir.dt.float16 if FP16 else mybir.dt.float32


def _build_core_inputs(x, W, U_rec, b, Wd):
    """Per-core numpy input dicts. Slot assignment: core0:(k0,k1), core1:(k2,k3),
    cores 2-7: (k4+i, pad)."""
    ndt = _np_dt()
    xt = np.ascontiguousarray(np.transpose(x, (2, 1, 0)).reshape(D, T * B))
    xaug = np.concatenate(
        [xt, np.ones((2, T * B), np.float32)], axis=0
    ).astype(ndt)

    slot_ks = [(0, 1), (2, 3)] + [(4 + i, None) for i in range(6)]
    neg_sc = -1.0 / SIG_BETA

    in_maps = []
    for core in range(NCORES):
        ks = slot_ks[core]
        LX = np.zeros((4, D_AUG, 2 * U), np.float32)  # [gate, 34, 128]
        LH = np.zeros((4, 2 * U, 2 * U), np.float32)  # [gate, 128, 128] blockdiag
        WD2 = np.zeros((2 * U, 1), np.float32)
        for s, k in enumerate(ks):
            if k is None:
                continue
            for g, gname in enumerate(_OUR_GATES):
                ref_g = _REF_GATE_SLICE[gname]
                cols = slice(ref_g * U, (ref_g + 1) * U)
                sc = 1.0 if gname == "c" else neg_sc
                LX[g, :D, s * U:(s + 1) * U] = W[k][:, cols] * sc
                LX[g, D, s * U:(s + 1) * U] = b[k][cols] * sc
                # exact +1.0 row for w = 1 - z/beta (c gate: no offset)
                LX[g, D + 1, s * U:(s + 1) * U] = 0.0 if gname == "c" else 1.0
                LH[g, s * U:(s + 1) * U, s * U:(s + 1) * U] = (
                    U_rec[k][:, cols] * sc
                )
            WD2[s * U:(s + 1) * U, 0] = Wd[:, 0]
        in_maps.append(
            {
                "xaug": xaug,
                "lx": np.ascontiguousarray(LX.astype(ndt)),
                "lh": np.ascontiguousarray(LH.astype(ndt)),
                "wd2": WD2.astype(ndt),
            }
        )
    return in_maps


def _build_program(t_steps: int) -> bacc.Bacc:
    from concourse import dve_ops

    POW256 = next(o for o in dve_ops.OPS if o.name == "DF_POW256_ANT")
    SIGR_MUL = next(o for o in dve_ops.OPS if o.name == "DF_SIGR_MUL_ANT")
    SIGR_MUL_RELU = next(
        o for o in dve_ops.OPS if o.name == "DF_SIGR_MUL_RELU_ANT"
    )

    nc = bacc.Bacc(
        "TRN2",
        target_bir_lowering=False,
        debug=False,
        enable_asserts=False,
        num_devices=NCORES,
    )
    MDT = _mm_dt()
    F32 = mybir.dt.float32
    xaug_ap = nc.dram_tensor("xaug", [D_AUG, T * B], MDT, kind="ExternalInput").ap()
    lx_ap = nc.dram_tensor("lx", [4, D_AUG, 2 * U], MDT, kind="ExternalInput").ap()
    lh_ap = nc.dram_tensor("lh", [4, 2 * U, 2 * U], MDT, kind="ExternalInput").ap()
    wd2_ap = nc.dram_tensor("wd2", [2 * U, 1], MDT, kind="ExternalInput").ap()
    y_ap = nc.dram_tensor("y", [B, t_steps], F32, kind="ExternalOutput").ap()

    P = 2 * U  # 128
    Bs = B // NS
    n_ybanks = (t_steps + 511) // 512

    with tile.TileContext(nc) as tc, ExitStack() as ctx:
        const_pool = ctx.enter_context(tc.tile_pool(name="const", bufs=1))
        state_pool = ctx.enter_context(tc.tile_pool(name="state", bufs=1))
        xch_pool = ctx.enter_context(tc.tile_pool(name="xch", bufs=2))
        z_pools = [
            ctx.enter_context(tc.tile_pool(name=f"z{s}", bufs=Z_BUFS, space="PSUM"))
            for s in range(NS)
        ]
        zc_pools = [
            ctx.enter_context(tc.tile_pool(name=f"zc{s}", bufs=Z_BUFS, space="PSUM"))
            for s in range(NS)
        ] if SPLIT_Z else None
        ypsum_pool = ctx.enter_context(tc.tile_pool(name="yps", bufs=1, space="PSUM"))
        v_pools = [
            ctx.enter_context(tc.tile_pool(name=f"v{s}", bufs=V_BUFS))
            for s in range(NS)
        ]
        vfo_pools = [
            ctx.enter_context(tc.tile_pool(name=f"vf{s}", bufs=V_BUFS))
            for s in range(NS)
        ]
        out_pool = ctx.enter_context(tc.tile_pool(name="out", bufs=1))

        # --- static weights into SBUF ---
        lx_tiles = []
        lh_tiles = []
        for g in range(4):
            lxg = const_pool.tile([D_AUG, P], MDT, tag=f"lx{g}", name=f"lxt{g}")
            nc.sync.dma_start(lxg[:], lx_ap[g])
            lx_tiles.append(lxg)
            lhg = const_pool.tile([P, P], MDT, tag=f"lh{g}", name=f"lht{g}")
            nc.sync.dma_start(lhg[:], lh_ap[g])
            lh_tiles.append(lhg)
        wd2 = const_pool.tile([P, 1], MDT, tag="wd2")
        nc.sync.dma_start(wd2[:], wd2_ap[:])

        # --- per-slice persistent state ---
        # h tiles are padded to 32 free columns (upper half stays zero from
        # the init memset): the y matmul then emits a 32-partition output,
        # which the walrus backend requires (16-partition outs crash it).
        HW_COLS = max(Bs, 32)
        hs = []   # [slice][phase]
        cs = []   # [slice]
        t1s = []
        t2s = []
        for s in range(NS):
            hps = []
            for ph in range(2 if H_DB else 1):
                t_ = state_pool.tile(
                    [P, HW_COLS], MDT, tag=f"h{s}_{ph}", name=f"h{s}_{ph}"
                )
                nc.vector.memset(t_[:], 0.0)
                hps.append(t_)
            hs.append(hps)
            c2 = state_pool.tile([P, Bs], F32, tag=f"c{s}", name=f"c{s}")
            nc.vector.memset(c2[:], 0.0)
            cs.append(c2)
            t1p = state_pool.tile([P, Bs], F32, tag=f"t1_{s}", name=f"t1_{s}")
            t2p = state_pool.tile([P, Bs], F32, tag=f"t2_{s}", name=f"t2_{s}")
            t1s.append(t1p)
            t2s.append(t2p)

        def h_read(s, t):
            return hs[s][(t + 1) % 2] if H_DB else hs[s][0]

        def h_write(s, t):
            return hs[s][t % 2] if H_DB else hs[s][0]

        # Per-slice y PSUM banks (matmul out base partition must be 0;
        # out partition count padded to HW_COLS=32 via the padded h tiles).
        ypsums = []
        for s in range(NS):
            yps = []
            for i in range(n_ybanks):
                yp = ypsum_pool.tile(
                    [HW_COLS, 512], F32, tag=f"yp{s}_{i}", name=f"ypt{s}_{i}"
                )
                yps.append(yp)
            ypsums.append(yps)

        def y_mm(s, tp):
            nc.tensor.matmul(
                ypsums[s][tp // 512][:, (tp % 512):(tp % 512) + 1],
                lhsT=h_write(s, tp)[:, 0:HW_COLS], rhs=wd2[:],
                start=True, stop=True,
            )

        def load_chunk(t):
            n_cols = min(CHUNK_STEPS, t_steps - t) * B
            xc = xch_pool.tile([D_AUG, CHUNK_STEPS * B], MDT, tag="xch")
            nc.sync.dma_start(xc[:, 0:n_cols], xaug_ap[:, t * B:t * B + n_cols])
            return xc

        xch = load_chunk(0)
        for t in range(t_steps):
            if t % CHUNK_STEPS == 0 and t > 0:
                xch = load_chunk(t)
            off = (t % CHUNK_STEPS) * B
            for s in range(NS):
                xrhs = xch[:, off + s * Bs: off + (s + 1) * Bs]
                hprev = h_read(s, t)
                # PSUM accumulation groups are bank-scoped: the start=True
                # x-mm and stop=True rec-mm of each gate must stay adjacent.
                nzg = 3 if SPLIT_Z else 4
                z_cur = z_pools[s].tile(
                    [P, nzg * Bs], F32, tag="z", name=f"z{s}_{t}"
                )
                if SPLIT_Z:
                    zc_t = zc_pools[s].tile(
                        [P, Bs], F32, tag="zc", name=f"zc{s}_{t}"
                    )
                for g in range(4):
                    if g < 3:
                        zg = z_cur[:, g * Bs:(g + 1) * Bs]
                    else:
                        zg = zc_t[:] if SPLIT_Z else z_cur[:, 3 * Bs:4 * Bs]
                    nc.tensor.matmul(
                        zg, lhsT=lx_tiles[g][:], rhs=xrhs,
                        start=True, stop=False, skip_group_check=True,
                    )
                    nc.tensor.matmul(
                        zg, lhsT=lh_tiles[g][:], rhs=hprev[:, 0:Bs],
                        start=False, stop=True, skip_group_check=True,
                    )
                zc_ap = zc_t[:] if SPLIT_Z else z_cur[:, 3 * Bs:4 * Bs]

                if Y_MM and t > 0:
                    y_mm(s, t - 1)

                # DVE block: v = w^256 over i|f|o, then fused gate ops.
                # Separate tiles for the split pows: a shared tile would
                # WAW-serialize them at tile granularity.
                if SPLIT_POW:
                    vi = v_pools[s].tile([P, Bs], F32, tag="vi", name=f"vi{s}_{t}")
                    vfo = vfo_pools[s].tile(
                        [P, 2 * Bs], F32, tag="vfo", name=f"vfo{s}_{t}"
                    )
                    nc.vector._custom_dve(POW256, out=vi[:], in0=z_cur[:, 0:Bs])
                    nc.vector._custom_dve(
                        POW256, out=vfo[:], in0=z_cur[:, Bs:3 * Bs]
                    )
                    v_t1, v_t2, v_h = vi[:, 0:Bs], vfo[:, 0:Bs], vfo[:, Bs:2 * Bs]
                else:
                    v = v_pools[s].tile([P, 3 * Bs], F32, tag="v", name=f"v{s}_{t}")
                    nc.vector._custom_dve(POW256, out=v[:], in0=z_cur[:, 0:3 * Bs])
                    v_t1, v_t2, v_h = v[:, 0:Bs], v[:, Bs:2 * Bs], v[:, 2 * Bs:3 * Bs]
                # t1 next: its PSUM-operand ack drains under t2's busy time
                nc.vector._custom_dve(
                    SIGR_MUL_RELU, out=t1s[s][:], in0=v_t1,
                    in1=zc_ap, s0=SIG_C0SEED, s1=SIG_TNR,
                )
                nc.vector._custom_dve(
                    SIGR_MUL, out=t2s[s][:], in0=v_t2, in1=cs[s][:],
                    s0=SIG_C0SEED, s1=SIG_TNR,
                )
                add_eng = nc.gpsimd if ADD_ENGINE == "gpsimd" else nc.vector
                add_eng.tensor_add(cs[s][:], t1s[s][:], t2s[s][:])
                nc.vector._custom_dve(
                    SIGR_MUL_RELU, out=h_write(s, t)[:, 0:Bs], in0=v_h,
                    in1=cs[s][:], s0=SIG_C0SEED, s1=SIG_TNR,
                )

        if Y_MM:
            for s in range(NS):
                y_mm(s, t_steps - 1)

        # Per-slice staging tiles at partition base 0: walrus crashes on
        # Activation writes to partition-offset SBUF APs, so each slice gets
        # its own tile and its own DMA to the right DRAM rows.
        for s in range(NS):
            ysb = out_pool.tile(
                [Bs, t_steps], F32, tag=f"ysb{s}", name=f"ysb{s}"
            )
            if Y_MM:
                for i in range(n_ybanks):
                    n = min(512, t_steps - i * 512)
                    nc.scalar.copy(
                        ysb[:, i * 512:i * 512 + n], ypsums[s][i][0:Bs, 0:n]
                    )
            else:
                nc.vector.memset(ysb[:], 0.0)
            nc.sync.dma_start(y_ap[s * Bs:(s + 1) * Bs, :], ysb[:])

    nc.compile()
    return nc


def kernel(x, W, U_rec, b, Wd, bd):
    x = np.asarray(x, np.float32)
    W = np.asarray(W, np.float32)
    U_rec = np.asarray(U_rec, np.float32)
    b = np.asarray(b, np.float32)
    Wd = np.asarray(Wd, np.float32)
    bd = np.asarray(bd, np.float32)

    in_maps = _build_core_inputs(x, W, U_rec, b, Wd)
    nc = _build_program(T)
    res = run_bass_kernel_spmd(nc, in_maps, core_ids=list(range(NCORES)))
    ysum = np.zeros((B, T), np.float64)
    for r in res.results:
        ysum += r["y"].astype(np.float64)
    y = (ysum / K + bd[0]).astype(np.float32)
    return y[:, :, None]


if __name__ == "__main__":
    rng = np.random.default_rng(0)
    out = kernel(
        rng.standard_normal((B, T, D), np.float32),
        rng.standard_normal((K, D, 4 * U), np.float32) * 0.05,
        rng.standard_normal((K, U, 4 * U), np.float32) * 0.05,
        np.zeros((K, 4 * U), np.float32),
        rng.standard_normal((U, 1), np.float32) * 0.05,
        np.zeros((1,), np.float32),
    )
    print(out.shape, out.dtype)

